# revision 46
# baseline (speedup 1.0000x reference)
"""CenterPooling (CornerNet) Trainium2 kernel — 8 NeuronCores.

Sharding: 8 cores = 4 batches x 2 H-halves.  Each core gets a host-padded
input slab [256, 70, 130] (3 halo rows each side, zero W-pad columns).

Key algebraic simplifications:
 - cummax(reverse) then cummax(forward) along an axis == global max along
   that axis, broadcast.  So the up branch only needs per-row maxes over W
   ([C, H]) and the down branch per-column maxes over H ([C, W]).
 - BN (eval mode) folds into conv weights/bias on the host; BN scale > 0 so
   max-reduction commutes with the affine+ReLU epilogue.
 - The merge conv's input is rank-structured: updown[c,h,w] = u[c,h] + d[c,w],
   so the 3x3 merge conv SEPARATES into tiny 1-D convs: an h-conv of u
   ([C, H] -> A(o,h), with 3 w-boundary classes of kx-summed weights) plus a
   w-conv of d ([C, W] -> B(o,w), with h-boundary corrections -Bk0/-Bk2 at
   the global top/bottom rows, applied data-driven via per-row selector
   vectors).  This removes the 18 big matmuls per merge block entirely.
 - Down-branch col-max needs a cross-half combine: pairwise AllReduce-max of
   a tiny [256, 128] tile.
 - H-pad semantics at the global top/bottom are handled data-driven (SPMD
   uniform program): a validity mask zeroes invalid u rows, and the per-row
   ACT bias adds -1e30 to out-of-range relu1 rows so ReLU clamps them to
   the zero-pad value.

The up/down convs run in fp8 e4m3 with DoubleRow perf mode: 9 shifted
matmuls per block, each contracting K=256 (both ci-tiles packed in the
k-subtile dim) — half the PE instructions of the bf16 version.  Their
max-pool epilogues attenuate the fp8 quantization noise (final rel err
~1% vs the 2% budget).  The c1/c2/merge matmuls stay bf16 (their error
lands directly on the output).  c2 blocks interleave with the relu1
assembly (3 blocks behind) so the PE stays busy through that phase.
"""

import sys

sys.path.insert(0, "/opt/trn_rl_repo")

import numpy as np
import ml_dtypes

import concourse.bacc as bacc
import concourse.tile as tile
import concourse.bass as bass
from concourse import mybir, bass_utils

BF16 = mybir.dt.bfloat16
F8 = mybir.dt.float8e4
F32 = mybir.dt.float32
NP_BF16 = ml_dtypes.bfloat16
NP_F8 = ml_dtypes.float8_e4m3
DR = mybir.MatmulPerfMode.DoubleRow

N_CORES = 8
B, CIN, C, H, W = 4, 256, 256, 128, 128
G = 3            # halo rows on each side of the 64 owned rows
HS = 64 + 2 * G  # 70 slab rows
WP = W + 2       # 130 (zero-pad col on each side)
EPS = 1e-5
NEG = -1e30

RELU = mybir.ActivationFunctionType.Relu
AX_X = mybir.AxisListType.X
ALU = mybir.AluOpType

_CACHE: dict = {}


def _mm_group(nc, ps_ap, mms, perf_mode=None):
    n = len(mms)
    for k, (lhsT, rhs) in enumerate(mms):
        nc.tensor.matmul(ps_ap, lhsT, rhs, start=(k == 0), stop=(k == n - 1),
                         perf_mode=perf_mode)


def _conv3_mms(wtile, src, s, nr, cot):
    """The 18 (ci,ky,kx) matmuls of a 3x3 conv block: output rows s..s+nr-1."""
    mms = []
    for cit in range(2):
        for ky in range(3):
            for kx in range(3):
                j = ((ky * 3 + kx) * 2 + cit) * 2 + cot
                mms.append((wtile[:, j, :], src[cit][:, s + ky - 1:s + ky - 1 + nr, kx:kx + W]))
    return mms


def _conv3_mms8(w8, xq, s, nr, cot):
    """The 9 DoubleRow (K=256) matmuls of a 3x3 fp8 conv block."""
    mms = []
    for ky in range(3):
        for kx in range(3):
            ko = ky * 3 + kx
            mms.append((w8[:, ko, :, cot * 128:(cot + 1) * 128],
                        xq[:, :, s + ky - 1:s + ky - 1 + nr, kx:kx + W]))
    return mms


def _build(inv_up, inv_dn):
    nc = bacc.Bacc("TRN2", target_bir_lowering=False, debug=False,
                   num_devices=N_CORES)

    x_d = nc.dram_tensor("x", [2, 128, HS, WP], BF16, kind="ExternalInput")
    xq_d = nc.dram_tensor("xq", [2, 128, HS, WP], F8, kind="ExternalInput")
    wup_d = nc.dram_tensor("wup8", [128, 9, 2, 256], F8, kind="ExternalInput")
    wdn_d = nc.dram_tensor("wdn8", [128, 9, 2, 256], F8, kind="ExternalInput")
    wc2_d = nc.dram_tensor("wc2w", [128, 48, 128], BF16, kind="ExternalInput")
    wc1_d = nc.dram_tensor("wc1", [128, 4, 128], BF16, kind="ExternalInput")
    wa_d = nc.dram_tensor("wa", [128, 36, 128], BF16, kind="ExternalInput")
    wb_d = nc.dram_tensor("wb", [128, 36, 128], BF16, kind="ExternalInput")
    bias_d = nc.dram_tensor("biases", [128, 8], F32, kind="ExternalInput")
    hv_d = nc.dram_tensor("hv", [128, HS], F32, kind="ExternalInput")
    pnegb_d = nc.dram_tensor("pnegb", [128, HS], F32, kind="ExternalInput")
    htop_d = nc.dram_tensor("htopneg", [128, HS], F32, kind="ExternalInput")
    hbot_d = nc.dram_tensor("hbotneg", [128, HS], F32, kind="ExternalInput")
    out_d = nc.dram_tensor("out", [2, 128, 64, W], F32, kind="ExternalOutput")

    with tile.TileContext(nc) as tc:
        with tc.tile_pool(name="const", bufs=1) as constp, \
             tc.tile_pool(name="acts", bufs=1) as actp, \
             tc.tile_pool(name="psum", bufs=6, space="PSUM") as psp, \
             tc.tile_pool(name="ostage", bufs=6) as osp, \
             tc.tile_pool(name="dram", bufs=1, space="DRAM") as dramp:

            wdn8 = constp.tile([128, 9, 2, 256], F8)
            for j0 in range(0, 9, 3):
                nc.sync.dma_start(wdn8[:, j0:j0 + 3, :, :], wdn_d.ap()[:, j0:j0 + 3, :, :])

            # fp8 input slab for the up/down convs, both ci-tiles packed for
            # DoubleRow (K=256) matmuls; interleaved row chunks so early rows
            # of both halves land first
            xq = actp.tile([128, 2, HS, WP], F8, name="xq")
            for r0 in range(0, HS, 8):
                r1_ = min(r0 + 8, HS)
                for cit in range(2):
                    nc.sync.dma_start(xq[:, cit, r0:r1_, :], xq_d.ap()[cit, :, r0:r1_, :])

            wup8 = constp.tile([128, 9, 2, 256], F8)
            nc.sync.dma_start(wup8[:, :, :, :], wup_d.ap())

            # bf16 slab only feeds the c1 1x1 conv
            xs = []
            for cit in range(2):
                xt = actp.tile([128, HS, WP], BF16, name=f"xs{cit}")
                xs.append(xt)
            for r0 in range(0, HS, 8):
                r1_ = min(r0 + 8, HS)
                for cit in range(2):
                    nc.sync.dma_start(xs[cit][:, r0:r1_, :], x_d.ap()[cit, :, r0:r1_, :])
            wc2w = constp.tile([128, 48, 128], BF16)
            nc.sync.dma_start(wc2w[:, :, :], wc2_d.ap())
            wc1 = constp.tile([128, 4, 128], BF16)
            nc.sync.dma_start(wc1[:, :, :], wc1_d.ap())
            wa = constp.tile([128, 36, 128], BF16)
            nc.sync.dma_start(wa[:, :, :], wa_d.ap())
            wb = constp.tile([128, 36, 128], BF16)
            nc.sync.dma_start(wb[:, :, :], wb_d.ap())
            biases = constp.tile([128, 8], F32)
            nc.sync.dma_start(biases[:, :], bias_d.ap())
            hv = constp.tile([128, HS], F32)
            nc.sync.dma_start(hv[:, :], hv_d.ap())
            pnegb = constp.tile([128, HS], F32)
            nc.sync.dma_start(pnegb[:, :], pnegb_d.ap())
            htopneg = constp.tile([128, HS], F32)
            nc.sync.dma_start(htopneg[:, :], htop_d.ap())
            hbotneg = constp.tile([128, HS], F32)
            nc.sync.dma_start(hbotneg[:, :], hbot_d.ap())

            # r1 is consumed by c2 only through its F(2,3) W-transform Tr;
            # keep just an 8-row ring of raw rows per ci-tile.
            r1 = []
            tr = []
            for cit in range(2):
                t2 = actp.tile([128, 8, WP], BF16, name=f"r1{cit}")
                nc.vector.memset(t2[:, :, 0], 0.0)
                nc.vector.memset(t2[:, :, WP - 1], 0.0)
                r1.append(t2)
                # Tr[cit][:, p, r, t]: transform point p of slab row r+2
                tr.append(actp.tile([128, 4, 66, 64], BF16, name=f"tr{cit}"))

            uraw, ufin, dacc, dmax, dfin = [], [], [], [], []
            for cot in range(2):
                t = actp.tile([128, HS], F32, name=f"uraw{cot}")
                nc.vector.memset(t[:, :], 0.0)
                uraw.append(t)
                ufin.append(actp.tile([128, HS], F32, name=f"ufin{cot}"))
                t = actp.tile([128, 4, W], F32, name=f"dacc{cot}")
                nc.vector.memset(t[:, :, :], -3e38)
                dacc.append(t)
                dmax.append(actp.tile([128, W], F32, name=f"dmax{cot}"))
                dfin.append(actp.tile([128, W], F32, name=f"dfin{cot}"))

            # ---- down branch: conv over the 64 owned rows, col-max over H ----
            for i in range(16):
                s = G + 4 * i
                for cot in range(2):
                    ps = psp.tile([128, 4, 128], F32, tag="ps", name="ps_dn", bufs=2)
                    _mm_group(nc, ps[:, :, :], _conv3_mms8(wdn8, xq, s, 4, cot),
                              perf_mode=DR)
                    nc.vector.tensor_max(dacc[cot][:, :, :], dacc[cot][:, :, :],
                                         ps[:, :, :])

            # pairwise (same-batch) AllReduce-max to get the global col-max
            cc_in = dramp.tile([256, W], F32)
            cc_out = dramp.tile([256, W], F32)
            for cot in range(2):
                nc.vector.tensor_max(dacc[cot][:, 0, :], dacc[cot][:, 0, :],
                                     dacc[cot][:, 1, :])
                nc.vector.tensor_max(dacc[cot][:, 2, :], dacc[cot][:, 2, :],
                                     dacc[cot][:, 3, :])
                nc.vector.tensor_max(dacc[cot][:, 0, :], dacc[cot][:, 0, :],
                                     dacc[cot][:, 2, :])
                nc.sync.dma_start(cc_in[cot * 128:(cot + 1) * 128, :], dacc[cot][:, 0, :])
            nc.gpsimd.collective_compute(
                "AllReduce", ALU.max,
                replica_groups=[[0, 1], [2, 3], [4, 5], [6, 7]],
                ins=[cc_in.opt()], outs=[cc_out.opt()])
            for cot in range(2):
                nc.sync.dma_start(dmax[cot][:, :], cc_out[cot * 128:(cot + 1) * 128, :])
                nc.scalar.activation(dfin[cot][:, :], dmax[cot][:, :], RELU,
                                     bias=biases[:, 2 + cot:3 + cot], scale=inv_dn)

            # ---- up branch: conv over rows [1, 69), row-max over W ----
            for i in range(17):
                s = 1 + 4 * i
                for cot in range(2):
                    ps = psp.tile([128, 4, 128], F32, tag="ps", name="ps_up", bufs=2)
                    _mm_group(nc, ps[:, :, :], _conv3_mms8(wup8, xq, s, 4, cot),
                              perf_mode=DR)
                    nc.vector.reduce_max(uraw[cot][:, s:s + 4], ps[:, :, :], axis=AX_X)
            for cot in range(2):
                nc.scalar.activation(ufin[cot][:, :], uraw[cot][:, :], RELU,
                                     bias=biases[:, cot:cot + 1], scale=inv_up)

            # ---- separable merge conv pieces ----
            # umask = u * hvalid (bf16), dpad = d with zero W-pad cols (bf16)
            umask, dpad = [], []
            for cit in range(2):
                t = actp.tile([128, HS], BF16, name=f"umask{cit}")
                nc.vector.tensor_mul(t[:, :], ufin[cit][:, :], hv[:, :])
                umask.append(t)
                t = actp.tile([128, WP], BF16, name=f"dpad{cit}")
                nc.vector.memset(t[:, :], 0.0)
                nc.vector.tensor_copy(t[:, 1:W + 1], dfin[cit][:, :])
                dpad.append(t)

            # A_cls(o,h): 1-D h-conv of umask with kx-summed merge weights.
            # cls 0=M (interior w), 1=L (w=0), 2=R (w=127).  Rows [2, 68).
            NA = 64 + 2
            asb = [[None, None, None], [None, None, None]]
            for cls in range(3):
                for cot in range(2):
                    psa = psp.tile([128, NA], F32, tag="ps", name="ps_a", bufs=2)
                    mms = []
                    for cit in range(2):
                        for ky in range(3):
                            j = ((cls * 3 + ky) * 2 + cit) * 2 + cot
                            mms.append((wa[:, j, :], umask[cit][:, 1 + ky:1 + ky + NA]))
                    _mm_group(nc, psa[:, :], mms)
                    t = actp.tile([128, NA], F32, name=f"asb{cls}{cot}")
                    nc.scalar.copy(t[:, :], psa[:, :])
                    asb[cot][cls] = t
            # afull = A_M + bias_pc1 + pneg (ACT bias per relu1 row);
            # afdL/afdR = A_L - A_M / A_R - A_M (w-edge fixups, pre-ReLU).
            afull, afdl, afdr = [], [], []
            for cot in range(2):
                t = actp.tile([128, HS], F32, name=f"afull{cot}")
                nc.vector.scalar_tensor_tensor(
                    t[:, 2:2 + NA], asb[cot][0][:, :], biases[:, 4 + cot:5 + cot],
                    pnegb[:, 2:2 + NA], op0=ALU.add, op1=ALU.add)
                afull.append(t)
                t = actp.tile([128, HS], F32, name=f"afdl{cot}")
                nc.vector.tensor_sub(t[:, 2:2 + NA], asb[cot][1][:, :], asb[cot][0][:, :])
                afdl.append(t)
                t = actp.tile([128, HS], F32, name=f"afdr{cot}")
                nc.vector.tensor_sub(t[:, 2:2 + NA], asb[cot][2][:, :], asb[cot][0][:, :])
                afdr.append(t)

            # B_var(o,w): 1-D w-conv of dpad with ky-summed merge weights.
            # var 0=M (all ky), 1=ky0 only, 2=ky2 only (boundary corrections).
            bt = [[None, None, None], [None, None, None]]
            for var in range(3):
                for cot in range(2):
                    psb = psp.tile([128, 128], F32, tag="ps", name="ps_b", bufs=2)
                    mms = []
                    for cit in range(2):
                        for kx in range(3):
                            j = ((var * 3 + kx) * 2 + cit) * 2 + cot
                            mms.append((wb[:, j, :], dpad[cit][:, kx:kx + W]))
                    _mm_group(nc, psb[:, :], mms)
                    t = actp.tile([128, 128], F32, name=f"bt{var}{cot}")
                    nc.scalar.copy(t[:, :], psb[:, :])
                    bt[cot][var] = t

            # ---- output conv: F(2,3) Winograd over 8-row superblocks.
            # N=512 moving keeps full PE rate; the 4 transform points split
            # across two 2-bank PSUM tiles so the first tile's readers can
            # release it while the second tile's GEMMs still run. ----
            def c2_super(k, cot):
                s = G + 8 * k
                if True:
                    # one single-bank PSUM tile per transform point, 6-deep
                    # ring: each point's tile frees right after its reader
                    # ops, so the next superblock's GEMMs never wait long
                    pp = []
                    for p in range(4):
                        pst = psp.tile([128, 8, 64], F32, tag="ps2",
                                       name="ps_c2", bufs=6)
                        mms = []
                        for cit in range(2):
                            for ky in range(3):
                                j = ((p * 3 + ky) * 2 + cit) * 2 + cot
                                mms.append((wc2w[:, j, :],
                                            tr[cit][:, p, s + ky - 3:s + ky + 5, :]))
                        _mm_group(nc, pst[:, :, :], mms)
                        pp.append(pst)
                    # even w = M0+M1+M2, odd w = M1-M2-M3; GPSIMD can't read
                    # PSUM and DVE ops take at most one PSUM operand, so M1
                    # and M3 go through SBUF via ACT copies.
                    c1s = osp.tile([128, 8, 64], F32, name="c1s", bufs=3)
                    c3s = osp.tile([128, 8, 64], F32, name="c3s", bufs=3)
                    nc.scalar.copy(c1s[:, :, :], pp[1][:, :, :])
                    nc.scalar.copy(c3s[:, :, :], pp[3][:, :, :])
                    ot = osp.tile([128, 8, 128], F32, name="ot", bufs=3)
                    nc.vector.tensor_add(ot[:, :, 0:128:2], pp[0][:, :, :], c1s[:, :, :])
                    nc.vector.tensor_add(ot[:, :, 0:128:2], ot[:, :, 0:128:2], pp[2][:, :, :])
                    nc.vector.tensor_sub(ot[:, :, 1:128:2], c1s[:, :, :], pp[2][:, :, :])
                    nc.gpsimd.tensor_sub(ot[:, :, 1:128:2], ot[:, :, 1:128:2], c3s[:, :, :])
                    nc.scalar.activation(ot[:, :, :], ot[:, :, :], RELU,
                                         bias=biases[:, 6 + cot:7 + cot], scale=1.0)
                    if k == 7:
                        # split the tail stores across rings so the last
                        # store's serial latency is halved
                        nc.sync.dma_start(out_d.ap()[cot, :, s - G:s - G + 4, :], ot[:, 0:4, :])
                        nc.sync.dma_start(out_d.ap()[cot, :, s - G + 4:s - G + 8, :], ot[:, 4:8, :])
                    else:
                        nc.sync.dma_start(out_d.ap()[cot, :, s - G:s - G + 8, :], ot[:, :, :])

            # ---- relu1 = relu(c1(x) + A + B + bias), assembled per block;
            # c2 blocks interleave two blocks behind so the PE always has
            # conv work queued, and their PSUM-releasing epilogue ops land
            # at the head of the ACT/DVE queues ----
            blocks = [(2 + 4 * i, 4) for i in range(16)] + [(66, 2)]
            for bi, (s, nr) in enumerate(blocks):
                # c2 superblock k consumes tr rows <= 8k+9, available once
                # relu1 block 2k+2 is transformed.  The two cot halves split
                # across consecutive iterations so every iteration carries
                # ~5.5us of covering PE work for the transform chains, and
                # each half's PSUM-releasing epilogue ops land at the head
                # of the ACT/DVE queues.
                if bi >= 3 and bi % 2 == 1:
                    c2_super((bi - 3) // 2, 0)
                elif bi >= 4 and bi % 2 == 0:
                    c2_super((bi - 4) // 2, 1)
                rr0 = (s - 2) % 8
                for cot in range(2):
                    ps = psp.tile([128, nr, 128], F32, tag="ps", name="ps_p", bufs=2)
                    mms = []
                    for cit in range(2):
                        mms.append((wc1[:, cit * 2 + cot, :], xs[cit][:, s:s + nr, 1:W + 1]))
                    _mm_group(nc, ps[:, :, :], mms)
                    for r in range(nr):
                        sr = s + r
                        # + B_M + (afull: A_M + bias + row mask) in one op
                        nc.vector.scalar_tensor_tensor(
                            ps[:, r, :], bt[cot][0][:, :], afull[cot][:, sr:sr + 1],
                            ps[:, r, :], op0=ALU.add, op1=ALU.add)
                        # global top/bottom boundary corrections live at fixed
                        # slab rows (G and HS-G-1); the selector data zeroes
                        # them on the half where they don't apply.
                        if sr == G:
                            nc.vector.scalar_tensor_tensor(
                                ps[:, r, :], bt[cot][1][:, :], htopneg[:, sr:sr + 1],
                                ps[:, r, :], op0=ALU.mult, op1=ALU.add)
                        if sr == HS - G - 1:
                            nc.vector.scalar_tensor_tensor(
                                ps[:, r, :], bt[cot][2][:, :], hbotneg[:, sr:sr + 1],
                                ps[:, r, :], op0=ALU.mult, op1=ALU.add)
                    nc.vector.tensor_add(ps[:, :, 0], ps[:, :, 0], afdl[cot][:, s:s + nr])
                    nc.vector.tensor_add(ps[:, :, W - 1], ps[:, :, W - 1], afdr[cot][:, s:s + nr])
                    nc.scalar.activation(r1[cot][:, rr0:rr0 + nr, 1:W + 1], ps[:, :, :],
                                         RELU, bias=0.0, scale=1.0)
                # F(2,3) input transform of the freshly assembled rows
                tb = s - 2
                for cit in range(2):
                    src = r1[cit]
                    d0 = src[:, rr0:rr0 + nr, 0:128:2]
                    d1 = src[:, rr0:rr0 + nr, 1:129:2]
                    d2 = src[:, rr0:rr0 + nr, 2:130:2]
                    d3 = src[:, rr0:rr0 + nr, 3:130:2]
                    nc.gpsimd.tensor_sub(tr[cit][:, 0, tb:tb + nr, :], d0, d2)
                    nc.gpsimd.tensor_add(tr[cit][:, 1, tb:tb + nr, :], d1, d2)
                    nc.gpsimd.tensor_sub(tr[cit][:, 2, tb:tb + nr, :], d2, d1)
                    nc.gpsimd.tensor_sub(tr[cit][:, 3, tb:tb + nr, :], d1, d3)
            c2_super(7, 0)
            c2_super(7, 1)

    nc.compile()
    return nc


def _pack3(w):
    # [256o, 256i, 3, 3] -> [128ci, j, 128co], j = ((ky*3+kx)*2+cit)*2+cot
    a = w.reshape(2, 128, 2, 128, 3, 3).transpose(3, 4, 5, 2, 0, 1)
    return np.ascontiguousarray(a.reshape(128, 36, 128)).astype(NP_BF16)


def _wscale(w):
    # power-of-2 scale putting max|w| near the top of the fp8e4m3 range
    return float(2.0 ** np.floor(np.log2(192.0 / np.abs(w).max())))


def _pack3_8(w, s):
    # [256o, 256i, 3, 3] -> [128p, 9ko, 2cit, 256o] fp8, scaled by s
    b = w.reshape(256, 2, 128, 3, 3)          # [o, cit, p, ky, kx]
    c = b.transpose(2, 3, 4, 1, 0)            # [p, ky, kx, cit, o]
    return np.ascontiguousarray(c.reshape(128, 9, 2, 256) * s).astype(NP_F8)


def _pack3_wino(w):
    # F(2,3) kx-transformed weights Gg: [256o,256i,3,3] ->
    # [128ci, j, 128co], j = ((p*3+ky)*2+cit)*2+cot
    g0, g1, g2 = w[:, :, :, 0], w[:, :, :, 1], w[:, :, :, 2]
    gg = np.stack([g0, (g0 + g1 + g2) * 0.5, (g0 - g1 + g2) * 0.5, g2])
    a = gg.reshape(4, 2, 128, 2, 128, 3)      # [p, cot, co, cit, ci, ky]
    b = a.transpose(4, 0, 5, 3, 1, 2)         # [ci, p, ky, cit, cot, co]
    return np.ascontiguousarray(b.reshape(128, 48, 128)).astype(NP_BF16)


def _pack1(w):
    # [256o, 256i, 1, 1] -> [128ci, j, 128co], j = cit*2+cot
    a = w[:, :, 0, 0].reshape(2, 128, 2, 128).transpose(3, 2, 0, 1)
    return np.ascontiguousarray(a.reshape(128, 4, 128)).astype(NP_BF16)


def _pack_sep(wk3):
    # wk3: [256o, 256i, 3] (kx- or ky-summed variants stacked on axis -1 by
    # caller as a dict) -> packs a [3var/cls, 3k, 256, 256] stack into
    # [128ci, j, 128co], j = ((v*3+k)*2+cit)*2+cot
    a = wk3.reshape(3, 3, 2, 128, 2, 128).transpose(5, 0, 1, 4, 2, 3)
    # dims now [ci, v, k, cit, cot, co]
    return np.ascontiguousarray(a.reshape(128, 36, 128)).astype(NP_BF16)


def _prep_in_maps(inputs):
    x = np.asarray(inputs["x"], dtype=np.float32)

    fw, fb = {}, {}
    for n in ["up", "down", "p", "c1", "c2"]:
        g = np.asarray(inputs[f"g_{n}"], np.float32)
        v = np.asarray(inputs[f"v_{n}"], np.float32)
        m = np.asarray(inputs[f"m_{n}"], np.float32)
        b = np.asarray(inputs[f"b_{n}"], np.float32)
        w = np.asarray(inputs[f"w_{n}"], np.float32)
        s = g / np.sqrt(v + EPS)
        fw[n] = w * s[:, None, None, None]
        fb[n] = b - m * s

    wp = fw["p"]
    wa_stack = np.stack([
        np.stack([wp[:, :, ky, :].sum(-1) for ky in range(3)]),            # M
        np.stack([wp[:, :, ky, 1:].sum(-1) for ky in range(3)]),           # L (w=0)
        np.stack([wp[:, :, ky, :2].sum(-1) for ky in range(3)]),           # R (w=127)
    ])
    wb_stack = np.stack([
        np.stack([wp[:, :, :, kx].sum(-1) for kx in range(3)]),            # M
        np.stack([wp[:, :, 0, kx] for kx in range(3)]),                    # ky=0
        np.stack([wp[:, :, 2, kx] for kx in range(3)]),                    # ky=2
    ])
    s_up = _wscale(fw["up"])
    s_dn = _wscale(fw["down"])
    consts = {
        "wup8": _pack3_8(fw["up"], s_up),
        "wdn8": _pack3_8(fw["down"], s_dn),
        "wc2w": _pack3_wino(fw["c2"]),
        "wc1": _pack1(fw["c1"]),
        "wa": _pack_sep(wa_stack),
        "wb": _pack_sep(wb_stack),
    }
    bias_np = np.zeros((128, 8), np.float32)
    for k, arr in enumerate([fb["up"], fb["down"], fb["p"] + fb["c1"], fb["c2"]]):
        m2 = arr.reshape(2, 128)
        bias_np[:, 2 * k] = m2[0]
        bias_np[:, 2 * k + 1] = m2[1]
    consts["biases"] = bias_np

    def _bcast(row):
        return np.ascontiguousarray(
            np.broadcast_to(row.astype(np.float32)[None, :], (128, HS)))

    in_maps = []
    for core in range(N_CORES):
        b_i, half = core // 2, core % 2
        slab = np.zeros((256, HS, WP), np.float32)
        if half == 0:
            slab[:, G:, 1:W + 1] = x[b_i][:, 0:HS - G, :]
            hv_row = (np.arange(HS) >= G)
            top_s, bot_s = G, None            # slab row of global row 0
        else:
            slab[:, :HS - G, 1:W + 1] = x[b_i][:, H - (HS - G):H, :]
            hv_row = (np.arange(HS) <= HS - G - 1)
            top_s, bot_s = None, HS - G - 1   # slab row of global row H-1
        slab2 = np.ascontiguousarray(slab.reshape(2, 128, HS, WP))
        xsl = slab2.astype(NP_BF16)
        xq8 = slab2.astype(NP_F8)
        pneg_row = np.where(hv_row, 0.0, NEG)
        htop_row = np.zeros(HS)
        if top_s is not None:
            htop_row[top_s] = -1.0
        hbot_row = np.zeros(HS)
        if bot_s is not None:
            hbot_row[bot_s] = -1.0
        in_maps.append({
            "x": xsl, "xq": xq8, "hv": _bcast(hv_row), "pnegb": _bcast(pneg_row),
            "htopneg": _bcast(htop_row), "hbotneg": _bcast(hbot_row), **consts})
    return in_maps, 1.0 / s_up, 1.0 / s_dn


def _run(inputs, trace=False):
    # Build a fresh Bass program per call: re-executing an already-loaded
    # NEFF on these cores intermittently trips NRT_EXEC_UNIT_UNRECOVERABLE,
    # while a fresh build+load is reliable (neuronxcc cache keeps it fast).
    in_maps, inv_up, inv_dn = _prep_in_maps(inputs)
    nc = _build(inv_up, inv_dn)
    res = bass_utils.run_bass_kernel_spmd(
        nc, in_maps, core_ids=list(range(N_CORES)), trace=trace)
    out = np.empty((B, C, H, W), np.float32)
    for core in range(N_CORES):
        b_i, half = core // 2, core % 2
        r = np.asarray(res.results[core]["out"]).reshape(256, 64, W)
        out[b_i, :, half * 64:(half + 1) * 64, :] = r
    return out, res


def kernel(**inputs) -> np.ndarray:
    out, _ = _run(inputs, trace=False)
    return out



# revision 47
# speedup vs baseline: 1.0634x; 1.0634x over previous
"""CenterPooling (CornerNet) Trainium2 kernel — 8 NeuronCores.

Sharding: 8 cores = 4 batches x 2 H-halves.  Each core gets a host-padded
input slab [256, 70, 130] (3 halo rows each side, zero W-pad columns).

Key algebraic simplifications:
 - cummax(reverse) then cummax(forward) along an axis == global max along
   that axis, broadcast.  So the up branch only needs per-row maxes over W
   ([C, H]) and the down branch per-column maxes over H ([C, W]).
 - BN (eval mode) folds into conv weights/bias on the host; BN scale > 0 so
   max-reduction commutes with the affine+ReLU epilogue.
 - The merge conv's input is rank-structured: updown[c,h,w] = u[c,h] + d[c,w],
   so the 3x3 merge conv SEPARATES into tiny 1-D convs: an h-conv of u
   ([C, H] -> A(o,h), with 3 w-boundary classes of kx-summed weights) plus a
   w-conv of d ([C, W] -> B(o,w), with h-boundary corrections -Bk0/-Bk2 at
   the global top/bottom rows, applied data-driven via per-row selector
   vectors).  This removes the 18 big matmuls per merge block entirely.
 - Down-branch col-max needs a cross-half combine: pairwise AllReduce-max of
   a tiny [256, 128] tile.
 - H-pad semantics at the global top/bottom are handled data-driven (SPMD
   uniform program): a validity mask zeroes invalid u rows, and the per-row
   ACT bias adds -1e30 to out-of-range relu1 rows so ReLU clamps them to
   the zero-pad value.

The up/down convs run in fp8 e4m3 with DoubleRow perf mode: 9 shifted
matmuls per block, each contracting K=256 (both ci-tiles packed in the
k-subtile dim) — half the PE instructions of the bf16 version.  Their
max-pool epilogues attenuate the fp8 quantization noise (final rel err
~1% vs the 2% budget).  The c1/c2/merge matmuls stay bf16 (their error
lands directly on the output).  c2 blocks interleave with the relu1
assembly (3 blocks behind) so the PE stays busy through that phase.
"""

import sys

sys.path.insert(0, "/opt/trn_rl_repo")

import numpy as np
import ml_dtypes

import concourse.bacc as bacc
import concourse.tile as tile
import concourse.bass as bass
from concourse import mybir, bass_utils

BF16 = mybir.dt.bfloat16
F8 = mybir.dt.float8e4
F32 = mybir.dt.float32
NP_BF16 = ml_dtypes.bfloat16
NP_F8 = ml_dtypes.float8_e4m3
DR = mybir.MatmulPerfMode.DoubleRow

N_CORES = 8
B, CIN, C, H, W = 4, 256, 256, 128, 128
G = 3            # halo rows on each side of the 64 owned rows
HS = 64 + 2 * G  # 70 slab rows
WP = W + 2       # 130 (zero-pad col on each side)
EPS = 1e-5
NEG = -1e30

RELU = mybir.ActivationFunctionType.Relu
AX_X = mybir.AxisListType.X
ALU = mybir.AluOpType

_CACHE: dict = {}


def _mm_group(nc, ps_ap, mms, perf_mode=None):
    n = len(mms)
    for k, (lhsT, rhs) in enumerate(mms):
        nc.tensor.matmul(ps_ap, lhsT, rhs, start=(k == 0), stop=(k == n - 1),
                         perf_mode=perf_mode)


def _conv3_mms(wtile, src, s, nr, cot):
    """The 18 (ci,ky,kx) matmuls of a 3x3 conv block: output rows s..s+nr-1."""
    mms = []
    for cit in range(2):
        for ky in range(3):
            for kx in range(3):
                j = ((ky * 3 + kx) * 2 + cit) * 2 + cot
                mms.append((wtile[:, j, :], src[cit][:, s + ky - 1:s + ky - 1 + nr, kx:kx + W]))
    return mms


def _conv3_mms8(w8, xq, s, nr, cot):
    """The 9 DoubleRow (K=256) matmuls of a 3x3 fp8 conv block."""
    mms = []
    for ky in range(3):
        for kx in range(3):
            ko = ky * 3 + kx
            mms.append((w8[:, ko, :, cot * 128:(cot + 1) * 128],
                        xq[:, :, s + ky - 1:s + ky - 1 + nr, kx:kx + W]))
    return mms


def _build(inv_up, inv_dn):
    nc = bacc.Bacc("TRN2", target_bir_lowering=False, debug=False,
                   num_devices=N_CORES)

    x_d = nc.dram_tensor("x", [2, 128, HS, WP], BF16, kind="ExternalInput")
    xq_d = nc.dram_tensor("xq", [2, 128, HS, WP], F8, kind="ExternalInput")
    wup_d = nc.dram_tensor("wup8", [128, 9, 2, 256], F8, kind="ExternalInput")
    wdn_d = nc.dram_tensor("wdn8", [128, 9, 2, 256], F8, kind="ExternalInput")
    wc2_d = nc.dram_tensor("wc2", [128, 36, 128], BF16, kind="ExternalInput")
    wc1_d = nc.dram_tensor("wc1", [128, 4, 128], BF16, kind="ExternalInput")
    wa_d = nc.dram_tensor("wa", [128, 36, 128], BF16, kind="ExternalInput")
    wb_d = nc.dram_tensor("wb", [128, 36, 128], BF16, kind="ExternalInput")
    bias_d = nc.dram_tensor("biases", [128, 8], F32, kind="ExternalInput")
    hv_d = nc.dram_tensor("hv", [128, HS], F32, kind="ExternalInput")
    pnegb_d = nc.dram_tensor("pnegb", [128, HS], F32, kind="ExternalInput")
    htop_d = nc.dram_tensor("htopneg", [128, HS], F32, kind="ExternalInput")
    hbot_d = nc.dram_tensor("hbotneg", [128, HS], F32, kind="ExternalInput")
    out_d = nc.dram_tensor("out", [2, 128, 64, W], F32, kind="ExternalOutput")

    with tile.TileContext(nc) as tc:
        with tc.tile_pool(name="const", bufs=1) as constp, \
             tc.tile_pool(name="acts", bufs=1) as actp, \
             tc.tile_pool(name="psum", bufs=6, space="PSUM") as psp, \
             tc.tile_pool(name="ostage", bufs=6) as osp, \
             tc.tile_pool(name="dram", bufs=1, space="DRAM") as dramp:

            wdn8 = constp.tile([128, 9, 2, 256], F8)
            for j0 in range(0, 9, 3):
                nc.sync.dma_start(wdn8[:, j0:j0 + 3, :, :], wdn_d.ap()[:, j0:j0 + 3, :, :])

            # fp8 input slab for the up/down convs, both ci-tiles packed for
            # DoubleRow (K=256) matmuls; interleaved row chunks so early rows
            # of both halves land first
            xq = actp.tile([128, 2, HS, WP], F8, name="xq")
            for r0 in range(0, HS, 8):
                r1_ = min(r0 + 8, HS)
                for cit in range(2):
                    nc.sync.dma_start(xq[:, cit, r0:r1_, :], xq_d.ap()[cit, :, r0:r1_, :])

            wup8 = constp.tile([128, 9, 2, 256], F8)
            nc.sync.dma_start(wup8[:, :, :, :], wup_d.ap())

            # bf16 slab only feeds the c1 1x1 conv
            xs = []
            for cit in range(2):
                xt = actp.tile([128, HS, WP], BF16, name=f"xs{cit}")
                xs.append(xt)
            for r0 in range(0, HS, 8):
                r1_ = min(r0 + 8, HS)
                for cit in range(2):
                    nc.sync.dma_start(xs[cit][:, r0:r1_, :], x_d.ap()[cit, :, r0:r1_, :])
            wc2 = constp.tile([128, 36, 128], BF16)
            nc.sync.dma_start(wc2[:, :, :], wc2_d.ap())
            wc1 = constp.tile([128, 4, 128], BF16)
            nc.sync.dma_start(wc1[:, :, :], wc1_d.ap())
            wa = constp.tile([128, 36, 128], BF16)
            nc.sync.dma_start(wa[:, :, :], wa_d.ap())
            wb = constp.tile([128, 36, 128], BF16)
            nc.sync.dma_start(wb[:, :, :], wb_d.ap())
            biases = constp.tile([128, 8], F32)
            nc.sync.dma_start(biases[:, :], bias_d.ap())
            hv = constp.tile([128, HS], F32)
            nc.sync.dma_start(hv[:, :], hv_d.ap())
            pnegb = constp.tile([128, HS], F32)
            nc.sync.dma_start(pnegb[:, :], pnegb_d.ap())
            htopneg = constp.tile([128, HS], F32)
            nc.sync.dma_start(htopneg[:, :], htop_d.ap())
            hbotneg = constp.tile([128, HS], F32)
            nc.sync.dma_start(hbotneg[:, :], hbot_d.ap())

            r1 = []
            for cit in range(2):
                t2 = actp.tile([128, HS, WP], BF16, name=f"r1{cit}")
                nc.vector.memset(t2[:, :, 0], 0.0)
                nc.vector.memset(t2[:, :, WP - 1], 0.0)
                r1.append(t2)

            uraw, ufin, dacc, dmax, dfin = [], [], [], [], []
            for cot in range(2):
                t = actp.tile([128, HS], F32, name=f"uraw{cot}")
                nc.vector.memset(t[:, :], 0.0)
                uraw.append(t)
                ufin.append(actp.tile([128, HS], F32, name=f"ufin{cot}"))
                t = actp.tile([128, 4, W], F32, name=f"dacc{cot}")
                nc.vector.memset(t[:, :, :], -3e38)
                dacc.append(t)
                dmax.append(actp.tile([128, W], F32, name=f"dmax{cot}"))
                dfin.append(actp.tile([128, W], F32, name=f"dfin{cot}"))

            # ---- down branch: conv over the 64 owned rows, col-max over H ----
            for i in range(16):
                s = G + 4 * i
                for cot in range(2):
                    ps = psp.tile([128, 4, 128], F32, tag="ps", name="ps_dn", bufs=3)
                    _mm_group(nc, ps[:, :, :], _conv3_mms8(wdn8, xq, s, 4, cot),
                              perf_mode=DR)
                    nc.vector.tensor_max(dacc[cot][:, :, :], dacc[cot][:, :, :],
                                         ps[:, :, :])

            # pairwise (same-batch) AllReduce-max to get the global col-max
            cc_in = dramp.tile([256, W], F32)
            cc_out = dramp.tile([256, W], F32)
            for cot in range(2):
                nc.vector.tensor_max(dacc[cot][:, 0, :], dacc[cot][:, 0, :],
                                     dacc[cot][:, 1, :])
                nc.vector.tensor_max(dacc[cot][:, 2, :], dacc[cot][:, 2, :],
                                     dacc[cot][:, 3, :])
                nc.vector.tensor_max(dacc[cot][:, 0, :], dacc[cot][:, 0, :],
                                     dacc[cot][:, 2, :])
                nc.sync.dma_start(cc_in[cot * 128:(cot + 1) * 128, :], dacc[cot][:, 0, :])
            nc.gpsimd.collective_compute(
                "AllReduce", ALU.max,
                replica_groups=[[0, 1], [2, 3], [4, 5], [6, 7]],
                ins=[cc_in.opt()], outs=[cc_out.opt()])
            for cot in range(2):
                nc.sync.dma_start(dmax[cot][:, :], cc_out[cot * 128:(cot + 1) * 128, :])
                nc.scalar.activation(dfin[cot][:, :], dmax[cot][:, :], RELU,
                                     bias=biases[:, 2 + cot:3 + cot], scale=inv_dn)

            # ---- up branch: conv over rows [1, 69), row-max over W ----
            for i in range(17):
                s = 1 + 4 * i
                for cot in range(2):
                    ps = psp.tile([128, 4, 128], F32, tag="ps", name="ps_up", bufs=3)
                    _mm_group(nc, ps[:, :, :], _conv3_mms8(wup8, xq, s, 4, cot),
                              perf_mode=DR)
                    nc.vector.reduce_max(uraw[cot][:, s:s + 4], ps[:, :, :], axis=AX_X)
            for cot in range(2):
                nc.scalar.activation(ufin[cot][:, :], uraw[cot][:, :], RELU,
                                     bias=biases[:, cot:cot + 1], scale=inv_up)

            # ---- separable merge conv pieces ----
            # umask = u * hvalid (bf16), dpad = d with zero W-pad cols (bf16)
            umask, dpad = [], []
            for cit in range(2):
                t = actp.tile([128, HS], BF16, name=f"umask{cit}")
                nc.vector.tensor_mul(t[:, :], ufin[cit][:, :], hv[:, :])
                umask.append(t)
                t = actp.tile([128, WP], BF16, name=f"dpad{cit}")
                nc.vector.memset(t[:, :], 0.0)
                nc.vector.tensor_copy(t[:, 1:W + 1], dfin[cit][:, :])
                dpad.append(t)

            # A_cls(o,h): 1-D h-conv of umask with kx-summed merge weights.
            # cls 0=M (interior w), 1=L (w=0), 2=R (w=127).  Rows [2, 68).
            NA = 64 + 2
            asb = [[None, None, None], [None, None, None]]
            for cls in range(3):
                for cot in range(2):
                    psa = psp.tile([128, NA], F32, tag="psa", name="ps_a", bufs=2)
                    mms = []
                    for cit in range(2):
                        for ky in range(3):
                            j = ((cls * 3 + ky) * 2 + cit) * 2 + cot
                            mms.append((wa[:, j, :], umask[cit][:, 1 + ky:1 + ky + NA]))
                    _mm_group(nc, psa[:, :], mms)
                    t = actp.tile([128, NA], F32, name=f"asb{cls}{cot}")
                    nc.scalar.copy(t[:, :], psa[:, :])
                    asb[cot][cls] = t
            # afull = A_M + bias_pc1 + pneg (ACT bias per relu1 row);
            # afdL/afdR = A_L - A_M / A_R - A_M (w-edge fixups, pre-ReLU).
            afull, afdl, afdr = [], [], []
            for cot in range(2):
                t = actp.tile([128, HS], F32, name=f"afull{cot}")
                nc.vector.scalar_tensor_tensor(
                    t[:, 2:2 + NA], asb[cot][0][:, :], biases[:, 4 + cot:5 + cot],
                    pnegb[:, 2:2 + NA], op0=ALU.add, op1=ALU.add)
                afull.append(t)
                t = actp.tile([128, HS], F32, name=f"afdl{cot}")
                nc.vector.tensor_sub(t[:, 2:2 + NA], asb[cot][1][:, :], asb[cot][0][:, :])
                afdl.append(t)
                t = actp.tile([128, HS], F32, name=f"afdr{cot}")
                nc.vector.tensor_sub(t[:, 2:2 + NA], asb[cot][2][:, :], asb[cot][0][:, :])
                afdr.append(t)

            # B_var(o,w): 1-D w-conv of dpad with ky-summed merge weights.
            # var 0=M (all ky), 1=ky0 only, 2=ky2 only (boundary corrections).
            bt = [[None, None, None], [None, None, None]]
            for var in range(3):
                for cot in range(2):
                    psb = psp.tile([128, 128], F32, tag="psa", name="ps_b", bufs=2)
                    mms = []
                    for cit in range(2):
                        for kx in range(3):
                            j = ((var * 3 + kx) * 2 + cit) * 2 + cot
                            mms.append((wb[:, j, :], dpad[cit][:, kx:kx + W]))
                    _mm_group(nc, psb[:, :], mms)
                    t = actp.tile([128, 128], F32, name=f"bt{var}{cot}")
                    nc.scalar.copy(t[:, :], psb[:, :])
                    bt[cot][var] = t

            # ---- output conv block (direct bf16) ----
            def c2_block(i):
                s = G + 4 * i
                for cot in range(2):
                    ps = psp.tile([128, 4, 128], F32, tag="ps2", name="ps_c2",
                                  bufs=3)
                    _mm_group(nc, ps[:, :, :], _conv3_mms(wc2, r1, s, 4, cot))
                    ot = osp.tile([128, 4, 128], F32, name="ot")
                    nc.scalar.activation(ot[:, :, :], ps[:, :, :], RELU,
                                         bias=biases[:, 6 + cot:7 + cot], scale=1.0)
                    if i >= 14:
                        nc.sync.dma_start(out_d.ap()[cot, :, s - G:s - G + 2, :], ot[:, 0:2, :])
                        nc.sync.dma_start(out_d.ap()[cot, :, s - G + 2:s - G + 4, :], ot[:, 2:4, :])
                    else:
                        nc.sync.dma_start(out_d.ap()[cot, :, s - G:s - G + 4, :], ot[:, :, :])

            # ---- relu1 = relu(c1(x) + A + B + bias), assembled per block;
            # c2 blocks interleave two blocks behind so the PE always has
            # conv work queued, and their PSUM-releasing epilogue ops land
            # at the head of the ACT/DVE queues ----
            blocks = [(2 + 4 * i, 4) for i in range(16)] + [(66, 2)]
            for bi, (s, nr) in enumerate(blocks):
                if bi >= 3:
                    c2_block(bi - 3)
                for cot in range(2):
                    ps = psp.tile([128, nr, 128], F32, tag="ps", name="ps_p", bufs=3)
                    mms = []
                    for cit in range(2):
                        mms.append((wc1[:, cit * 2 + cot, :], xs[cit][:, s:s + nr, 1:W + 1]))
                    _mm_group(nc, ps[:, :, :], mms)
                    for r in range(nr):
                        sr = s + r
                        nc.vector.scalar_tensor_tensor(
                            ps[:, r, :], bt[cot][0][:, :], afull[cot][:, sr:sr + 1],
                            ps[:, r, :], op0=ALU.add, op1=ALU.add)
                        if sr == G:
                            nc.vector.scalar_tensor_tensor(
                                ps[:, r, :], bt[cot][1][:, :], htopneg[:, sr:sr + 1],
                                ps[:, r, :], op0=ALU.mult, op1=ALU.add)
                        if sr == HS - G - 1:
                            nc.vector.scalar_tensor_tensor(
                                ps[:, r, :], bt[cot][2][:, :], hbotneg[:, sr:sr + 1],
                                ps[:, r, :], op0=ALU.mult, op1=ALU.add)
                    nc.vector.tensor_add(ps[:, :, 0], ps[:, :, 0], afdl[cot][:, s:s + nr])
                    nc.vector.tensor_add(ps[:, :, W - 1], ps[:, :, W - 1], afdr[cot][:, s:s + nr])
                    nc.scalar.activation(r1[cot][:, s:s + nr, 1:W + 1], ps[:, :, :],
                                         RELU, bias=0.0, scale=1.0)
            c2_block(14)
            c2_block(15)

    nc.compile()
    return nc


def _pack3(w):
    # [256o, 256i, 3, 3] -> [128ci, j, 128co], j = ((ky*3+kx)*2+cit)*2+cot
    a = w.reshape(2, 128, 2, 128, 3, 3).transpose(3, 4, 5, 2, 0, 1)
    return np.ascontiguousarray(a.reshape(128, 36, 128)).astype(NP_BF16)


def _wscale(w):
    # power-of-2 scale putting max|w| near the top of the fp8e4m3 range
    return float(2.0 ** np.floor(np.log2(192.0 / np.abs(w).max())))


def _pack3_8(w, s):
    # [256o, 256i, 3, 3] -> [128p, 9ko, 2cit, 256o] fp8, scaled by s
    b = w.reshape(256, 2, 128, 3, 3)          # [o, cit, p, ky, kx]
    c = b.transpose(2, 3, 4, 1, 0)            # [p, ky, kx, cit, o]
    return np.ascontiguousarray(c.reshape(128, 9, 2, 256) * s).astype(NP_F8)


def _pack3_wino(w):
    # F(2,3) kx-transformed weights Gg: [256o,256i,3,3] ->
    # [128ci, j, 128co], j = ((p*3+ky)*2+cit)*2+cot
    g0, g1, g2 = w[:, :, :, 0], w[:, :, :, 1], w[:, :, :, 2]
    gg = np.stack([g0, (g0 + g1 + g2) * 0.5, (g0 - g1 + g2) * 0.5, g2])
    a = gg.reshape(4, 2, 128, 2, 128, 3)      # [p, cot, co, cit, ci, ky]
    b = a.transpose(4, 0, 5, 3, 1, 2)         # [ci, p, ky, cit, cot, co]
    return np.ascontiguousarray(b.reshape(128, 48, 128)).astype(NP_BF16)


def _pack1(w):
    # [256o, 256i, 1, 1] -> [128ci, j, 128co], j = cit*2+cot
    a = w[:, :, 0, 0].reshape(2, 128, 2, 128).transpose(3, 2, 0, 1)
    return np.ascontiguousarray(a.reshape(128, 4, 128)).astype(NP_BF16)


def _pack_sep(wk3):
    # wk3: [256o, 256i, 3] (kx- or ky-summed variants stacked on axis -1 by
    # caller as a dict) -> packs a [3var/cls, 3k, 256, 256] stack into
    # [128ci, j, 128co], j = ((v*3+k)*2+cit)*2+cot
    a = wk3.reshape(3, 3, 2, 128, 2, 128).transpose(5, 0, 1, 4, 2, 3)
    # dims now [ci, v, k, cit, cot, co]
    return np.ascontiguousarray(a.reshape(128, 36, 128)).astype(NP_BF16)


def _prep_in_maps(inputs):
    x = np.asarray(inputs["x"], dtype=np.float32)

    fw, fb = {}, {}
    for n in ["up", "down", "p", "c1", "c2"]:
        g = np.asarray(inputs[f"g_{n}"], np.float32)
        v = np.asarray(inputs[f"v_{n}"], np.float32)
        m = np.asarray(inputs[f"m_{n}"], np.float32)
        b = np.asarray(inputs[f"b_{n}"], np.float32)
        w = np.asarray(inputs[f"w_{n}"], np.float32)
        s = g / np.sqrt(v + EPS)
        fw[n] = w * s[:, None, None, None]
        fb[n] = b - m * s

    wp = fw["p"]
    wa_stack = np.stack([
        np.stack([wp[:, :, ky, :].sum(-1) for ky in range(3)]),            # M
        np.stack([wp[:, :, ky, 1:].sum(-1) for ky in range(3)]),           # L (w=0)
        np.stack([wp[:, :, ky, :2].sum(-1) for ky in range(3)]),           # R (w=127)
    ])
    wb_stack = np.stack([
        np.stack([wp[:, :, :, kx].sum(-1) for kx in range(3)]),            # M
        np.stack([wp[:, :, 0, kx] for kx in range(3)]),                    # ky=0
        np.stack([wp[:, :, 2, kx] for kx in range(3)]),                    # ky=2
    ])
    s_up = _wscale(fw["up"])
    s_dn = _wscale(fw["down"])
    consts = {
        "wup8": _pack3_8(fw["up"], s_up),
        "wdn8": _pack3_8(fw["down"], s_dn),
        "wc2": _pack3(fw["c2"]),
        "wc1": _pack1(fw["c1"]),
        "wa": _pack_sep(wa_stack),
        "wb": _pack_sep(wb_stack),
    }
    bias_np = np.zeros((128, 8), np.float32)
    for k, arr in enumerate([fb["up"], fb["down"], fb["p"] + fb["c1"], fb["c2"]]):
        m2 = arr.reshape(2, 128)
        bias_np[:, 2 * k] = m2[0]
        bias_np[:, 2 * k + 1] = m2[1]
    consts["biases"] = bias_np

    def _bcast(row):
        return np.ascontiguousarray(
            np.broadcast_to(row.astype(np.float32)[None, :], (128, HS)))

    in_maps = []
    for core in range(N_CORES):
        b_i, half = core // 2, core % 2
        slab = np.zeros((256, HS, WP), np.float32)
        if half == 0:
            slab[:, G:, 1:W + 1] = x[b_i][:, 0:HS - G, :]
            hv_row = (np.arange(HS) >= G)
            top_s, bot_s = G, None            # slab row of global row 0
        else:
            slab[:, :HS - G, 1:W + 1] = x[b_i][:, H - (HS - G):H, :]
            hv_row = (np.arange(HS) <= HS - G - 1)
            top_s, bot_s = None, HS - G - 1   # slab row of global row H-1
        slab2 = np.ascontiguousarray(slab.reshape(2, 128, HS, WP))
        xsl = slab2.astype(NP_BF16)
        xq8 = slab2.astype(NP_F8)
        pneg_row = np.where(hv_row, 0.0, NEG)
        htop_row = np.zeros(HS)
        if top_s is not None:
            htop_row[top_s] = -1.0
        hbot_row = np.zeros(HS)
        if bot_s is not None:
            hbot_row[bot_s] = -1.0
        in_maps.append({
            "x": xsl, "xq": xq8, "hv": _bcast(hv_row), "pnegb": _bcast(pneg_row),
            "htopneg": _bcast(htop_row), "hbotneg": _bcast(hbot_row), **consts})
    return in_maps, 1.0 / s_up, 1.0 / s_dn


def _run(inputs, trace=False):
    # Build a fresh Bass program per call: re-executing an already-loaded
    # NEFF on these cores intermittently trips NRT_EXEC_UNIT_UNRECOVERABLE,
    # while a fresh build+load is reliable (neuronxcc cache keeps it fast).
    in_maps, inv_up, inv_dn = _prep_in_maps(inputs)
    nc = _build(inv_up, inv_dn)
    res = bass_utils.run_bass_kernel_spmd(
        nc, in_maps, core_ids=list(range(N_CORES)), trace=trace)
    out = np.empty((B, C, H, W), np.float32)
    for core in range(N_CORES):
        b_i, half = core // 2, core % 2
        r = np.asarray(res.results[core]["out"]).reshape(256, 64, W)
        out[b_i, :, half * 64:(half + 1) * 64, :] = r
    return out, res


def kernel(**inputs) -> np.ndarray:
    out, _ = _run(inputs, trace=False)
    return out

